# revision 51
# baseline (speedup 1.0000x reference)
"""Cross-attention with relative-position-bias MLP on 8 Trainium2 NeuronCores.

Sharding: batch-parallel attention (core c owns batch element c) +
k-sharded bias MLP: core c computes bias rows for keys
{g*64 + c*8 + j : g<8, j<8} so that the AllGather of chunk g yields a
DRAM layout [c, j, h, q] whose flattened (c, j) order IS the global key
order of the 64-key block g.  Phase 3 runs attention fully transposed
(logits as [k, q]): bias tiles stream in with 2KB descriptors (two
64-partition halves per 128-k tile) and are folded into the logits PSUM
via an fp16 identity-matmul accumulate; exp reads PSUM directly (bf16
output - range safe without max subtraction); AV consumes exp with V in
natural [k, dh] layout plus a ones column that yields the softmax sums
for free; per-head normalization is a bf16 reciprocal + K=1 bf16
broadcast matmul + one [64,512] multiply.

Precision: 16-bit everywhere on the PE (1 cyc/row; f32r lowers to
2-pass fp32 HIGH mode on this stack), fp32 PSUM accumulation. The bias
MLP mm1 uses bf16 hi/lo packed into K=24 (exact to ~2^-17).

Scheduling: phase-1 mm2 runs two dc slots late (software pipeline) and
phase-3 projections interleave one matmul per dc slot so the PE never
idles on the gelu chain (frequent short stalls halve the PE clock);
phase-3 heads run in a 2-stage pipeline so QK work of head h+1 covers
the exp latency of head h. Big weight loads ride the ACT/Pool DMA
queues so the Sync queue serves the latency-critical rel/shard/bias
tiles.

Self-contained: hardcodes all shapes; builds/compiles the Bass kernel on
first call and runs it via bass_utils.run_bass_kernel_spmd on cores 0-7.
"""

import numpy as np

import concourse.bass as bass
import concourse.mybir as mybir
import concourse.tile as tile
from concourse import bacc, bass_utils

F32 = mybir.dt.float32
F32R = mybir.dt.float32r
BF16 = mybir.dt.bfloat16
FP16 = mybir.dt.float16
AF = mybir.ActivationFunctionType
ADD = mybir.AluOpType.add
MULT = mybir.AluOpType.mult

NCORES = 8
B = 8
L = 512
D = 768
H = 12
DH = 64
NCH = D // 128
KS = L // NCORES          # 64 owned keys per core
NSTEP = KS // 2           # 32 phase-1 steps (2 owned keys per step)
NCHUNK = 4                # all-gather chunks (16 owned keys each)
SPC = NSTEP // NCHUNK     # steps per chunk = 4
SCALE = DH ** -0.5

_CACHE = {}


def _build(dbg=False):
    nc = bacc.Bacc("TRN2", target_bir_lowering=False, debug=False, num_devices=NCORES)

    xqT_d = nc.dram_tensor("xqT", [D, L], FP16, kind="ExternalInput")
    kvT_d = nc.dram_tensor("kvT", [D, L], FP16, kind="ExternalInput")
    kvTB_d = nc.dram_tensor("kvTB", [D, L], BF16, kind="ExternalInput")
    relP_d = nc.dram_tensor("relP", [128, NSTEP * 2 * L], BF16, kind="ExternalInput")
    WqS_d = nc.dram_tensor("WqS", [128, NCH, D], FP16, kind="ExternalInput")
    Wk_d = nc.dram_tensor("Wk", [128, NCH, D], FP16, kind="ExternalInput")
    Wv_d = nc.dram_tensor("Wv", [128, NCH, D], BF16, kind="ExternalInput")
    Wo_d = nc.dram_tensor("Wo", [128, H, D], FP16, kind="ExternalInput")
    W1P_d = nc.dram_tensor("W1P", [128, D], BF16, kind="ExternalInput")
    W2P_d = nc.dram_tensor("W2P", [128, NCH, H], FP16, kind="ExternalInput")
    bqS_d = nc.dram_tensor("bqS", [128, NCH], F32, kind="ExternalInput")
    bk_d = nc.dram_tensor("bk", [128, NCH], F32, kind="ExternalInput")
    b1_d = nc.dram_tensor("b1", [128, NCH], F32, kind="ExternalInput")
    b2_d = nc.dram_tensor("b2", [H, 1], F32, kind="ExternalInput")
    bv_d = nc.dram_tensor("bvb", [128, D], F32, kind="ExternalInput")
    bo_d = nc.dram_tensor("bob", [128, D], F32, kind="ExternalInput")
    idn_d = nc.dram_tensor("idn", [128, 128], FP16, kind="ExternalInput")
    one_d = nc.dram_tensor("one", [1, 64], BF16, kind="ExternalInput")
    zer_d = nc.dram_tensor("zer", [64, H * L], FP16, kind="ExternalInput")
    oneh_d = nc.dram_tensor("oneh", [128, 4 * H], BF16, kind="ExternalInput")
    out_d = nc.dram_tensor("out", [L, D], F32, kind="ExternalOutput")
    if dbg:
        dbg_full = nc.dram_tensor(
            "dbg_full", [NCORES, 16, H, L], FP16, kind="ExternalOutput"
        )
        dbg_qt = nc.dram_tensor("dbg_qt", [128, NCH, L], FP16, kind="ExternalOutput")
        dbg_exp = nc.dram_tensor("dbg_exp", [128, L], BF16, kind="ExternalOutput")
        dbg_att = nc.dram_tensor("dbg_att", [DH, H, L], FP16, kind="ExternalOutput")

    with tile.TileContext(nc) as tc:
        with (
            tc.tile_pool(name="dram", bufs=1, space="DRAM") as dpool,
            tc.tile_pool(name="persist", bufs=1) as pp,
        ):
            shards = [
                dpool.tile([16, H, L], FP16, name=f"shard{g}") for g in range(NCHUNK)
            ]
            fulls = [
                dpool.tile([NCORES, 16, H, L], FP16, name=f"full{g}",
                           addr_space="Shared")
                for g in range(NCHUNK)
            ]

            # ---- persistent SBUF (phase-1-critical small tiles on sync) ----
            W1p_sb = pp.tile([128, D], BF16, name="W1p_sb")
            nc.sync.dma_start(W1p_sb[:], W1P_d[:, :])
            W2P_sb = pp.tile([128, NCH, H], FP16, name="W2P_sb")
            nc.sync.dma_start(W2P_sb[:], W2P_d[:, :, :])
            b1_sb = pp.tile([128, NCH], F32, name="b1_sb")
            nc.sync.dma_start(b1_sb[:], b1_d[:, :])
            b2_sb = pp.tile([H, 1], F32, name="b2_sb")
            nc.sync.dma_start(b2_sb[:], b2_d[:, :])
            # rest of the constants ride the ACT-engine DMA queue
            bq_sb = pp.tile([128, NCH], F32, name="bq_sb")
            nc.scalar.dma_start(bq_sb[:], bqS_d[:, :])
            bk_sb = pp.tile([128, NCH], F32, name="bk_sb")
            nc.scalar.dma_start(bk_sb[:], bk_d[:, :])
            bv_sb = pp.tile([128, D], F32, name="bv_sb")
            nc.scalar.dma_start(bv_sb[:], bv_d[:, :])
            bo_sb = pp.tile([128, D], F32, name="bo_sb")
            nc.scalar.dma_start(bo_sb[:], bo_d[:, :])
            ident = pp.tile([128, 128], FP16, name="ident")
            nc.scalar.dma_start(ident[:], idn_d[:, :])
            ones1 = pp.tile([1, DH], BF16, name="ones1")
            nc.scalar.dma_start(ones1[:], one_d[0:1, :])

            qT_sb = pp.tile([128, NCH, L], FP16, name="qT_sb")
            kT_sb = pp.tile([128, NCH, L], FP16, name="kT_sb")
            # V in natural [k, dh] layout, 65 slots per head (slot 64 = ones)
            v_sb = pp.tile([128, 4, H, DH + 1], BF16, name="v_sb")
            oneh_sb = pp.tile([128, 4 * H], BF16, name="oneh_sb")
            nc.scalar.dma_start(oneh_sb[:], oneh_d[:, :])
            nc.scalar.activation(
                v_sb[:, :, :, DH:DH + 1].rearrange("p g h o -> p g (h o)"),
                oneh_sb[:].rearrange("p (g h) -> p g h", g=4),
                AF.Copy,
            )
            attnT = pp.tile([128, H, L], FP16, name="attnT")
            nc.scalar.dma_start(attnT[DH:128, :, :], zer_d.ap().rearrange(
                "p (h q) -> p h q", h=H))
            qTp = pp.tile([128, H, L], FP16, name="qTp")
            nc.scalar.dma_start(qTp[DH:128, :, :], zer_d.ap().rearrange(
                "p (h q) -> p h q", h=H))
            kTp = pp.tile([128, H, L], FP16, name="kTp")
            nc.scalar.dma_start(kTp[DH:128, :, :], zer_d.ap().rearrange(
                "p (h q) -> p h q", h=H))
            Wo_sb = pp.tile([128, H, D], FP16, name="Wo_sb")
            nc.scalar.dma_start(Wo_sb[:], Wo_d[:, :, :])

            # ---- phase 1 + interleaved projections ----
            with (
                tc.tile_pool(name="p1w", bufs=1) as p1w,
                tc.tile_pool(name="p1rel", bufs=5) as p1rel,
                tc.tile_pool(name="p1gel", bufs=4) as p1gel,
                tc.tile_pool(name="p1out", bufs=3) as p1out,
                tc.tile_pool(name="p1ps", bufs=2, space="PSUM") as p1ps,
                tc.tile_pool(name="p1psb", bufs=1, space="PSUM") as p1psb,
                tc.tile_pool(name="p1pj", bufs=2, space="PSUM") as p1pj,
            ):
                # activation-engine DMA queue: big projection operands
                WqS_sb = p1w.tile([128, NCH, D], FP16, name="WqS_sb")
                nc.scalar.dma_start(WqS_sb[:], WqS_d[:, :, :])
                xqT_sb = p1w.tile([128, NCH, L], FP16, name="xqT_sb")
                nc.scalar.dma_start(
                    xqT_sb[:], xqT_d.ap().rearrange("(c p) t -> p c t", p=128)
                )
                Wk_sb = p1w.tile([128, NCH, D], FP16, name="Wk_sb")
                nc.scalar.dma_start(Wk_sb[:], Wk_d[:, :, :])
                kvT_sb = p1w.tile([128, NCH, L], FP16, name="kvT_sb")
                nc.scalar.dma_start(
                    kvT_sb[:], kvT_d.ap().rearrange("(c p) t -> p c t", p=128)
                )
                Wv_sb = p1w.tile([128, NCH, D], BF16, name="Wv_sb")
                nc.scalar.dma_start(Wv_sb[:], Wv_d[:, :, :])
                kvTb_sb = p1w.tile([128, NCH, L], BF16, name="kvTb_sb")
                nc.scalar.dma_start(
                    kvTb_sb[:], kvTB_d.ap().rearrange("(c p) t -> p c t", p=128)
                )

                def proj_microops():
                    # q / k projections: out [128 (oc dims), 512]
                    for W_sb, x_sb, b_sb, out_t, pfx in (
                        (WqS_sb, xqT_sb, bq_sb, qT_sb, "q"),
                        (Wk_sb, kvT_sb, bk_sb, kT_sb, "k"),
                    ):
                        for oc in range(NCH):
                            cell = {}

                            def mk_mm(di, cell=cell, W_sb=W_sb, x_sb=x_sb,
                                      oc=oc, pfx=pfx):
                                def op():
                                    if di == 0:
                                        cell["ps"] = p1pj.tile(
                                            [128, L], F32, tag="pj",
                                            name=f"pp{pfx}_{oc}",
                                        )
                                    nc.tensor.matmul(
                                        cell["ps"][:],
                                        W_sb[:, di, oc * 128:(oc + 1) * 128],
                                        x_sb[:, di, :],
                                        start=(di == 0),
                                        stop=(di == NCH - 1),
                                    )
                                return op
                            for di in range(NCH):
                                yield mk_mm(di)

                            def fin(cell=cell, b_sb=b_sb, out_t=out_t, oc=oc):
                                nc.vector.tensor_scalar_add(
                                    out_t[:, oc, :], cell["ps"][:],
                                    b_sb[:, oc:oc + 1],
                                )
                            yield fin
                    # v projection: out [128 k-chunk, 6 heads x 64]
                    for tc4 in range(4):
                        for hf in range(2):
                            cell = {}

                            def mk_mm(di, cell=cell, tc4=tc4, hf=hf):
                                def op():
                                    if di == 0:
                                        cell["ps"] = p1pj.tile(
                                            [128, L], F32, tag="pj",
                                            name=f"ppv_{tc4}_{hf}",
                                        )
                                    nc.tensor.matmul(
                                        cell["ps"][:, 0:384],
                                        kvTb_sb[:, di,
                                                tc4 * 128:(tc4 + 1) * 128],
                                        Wv_sb[:, di, hf * 384:(hf + 1) * 384],
                                        start=(di == 0),
                                        stop=(di == NCH - 1),
                                    )
                                return op
                            for di in range(NCH):
                                yield mk_mm(di)

                            def fin(cell=cell, tc4=tc4, hf=hf):
                                nc.vector.tensor_tensor(
                                    v_sb[:, tc4, 6 * hf:6 * (hf + 1), 0:DH],
                                    cell["ps"][:, 0:384].rearrange(
                                        "p (h d) -> p h d", h=6
                                    ),
                                    bv_sb[
                                        :, hf * 384:(hf + 1) * 384
                                    ].rearrange("p (h d) -> p h d", h=6),
                                    op=ADD,
                                )
                            yield fin
                micro = proj_microops()
                micro_done = False

                from collections import deque
                pend = deque()
                bias_tiles = {}

                def finish_step(s):
                    g = s // SPC
                    bias_ps = bias_tiles.pop(s)
                    bsb = p1out.tile([H, 2 * L], FP16, tag="bsb",
                                     name=f"bsb_{s}")
                    nc.vector.tensor_scalar_add(bsb[:], bias_ps[:], b2_sb[:, 0:1])
                    j0 = (2 * s) % 16
                    nc.sync.dma_start(
                        shards[g][j0:j0 + 2, :, :].rearrange("k h q -> h k q"),
                        bsb[:].rearrange("h (k q) -> h k q", k=2),
                    )
                    if s % SPC == SPC - 1:
                        nc.gpsimd.collective_compute(
                            "AllGather",
                            mybir.AluOpType.bypass,
                            replica_groups=[list(range(NCORES))],
                            ins=[shards[g][:].opt()],
                            outs=[fulls[g][:].opt()],
                        )

                rels = {}

                def load_rel(s):
                    if s >= NSTEP:
                        return
                    rel2 = p1rel.tile([128, 2 * L], BF16, tag="rel",
                                      name=f"rel_{s}")
                    nc.sync.dma_start(
                        rel2[:], relP_d[:, s * 2 * L:(s + 1) * 2 * L]
                    )
                    rels[s] = rel2

                for s in range(4):
                    load_rel(s)
                for s in range(NSTEP):
                    load_rel(s + 4)
                    rel2 = rels.pop(s)
                    bias_ps = p1psb.tile([H, 2 * L], F32, tag="bps",
                                         name=f"bps_{s}")
                    bias_tiles[s] = bias_ps
                    for dc in range(NCH):
                        hid = p1ps.tile([128, 2 * L], F32, tag="hid",
                                        name=f"hid_{s}_{dc}")
                        for j in range(2):
                            nc.tensor.matmul(
                                hid[:, j * L:(j + 1) * L],
                                W1p_sb[:, dc * 128:(dc + 1) * 128],
                                rel2[:, j * L:(j + 1) * L],
                                start=True,
                                stop=True,
                            )
                        gelw = p1gel.tile([128, 2 * L], FP16, tag="gel",
                                          name=f"gel_{s}_{dc}")
                        nc.scalar.activation(
                            gelw[:], hid[:], AF.Gelu, bias=b1_sb[:, dc:dc + 1]
                        )

                        def mm2(s=s, dc=dc, bias_ps=bias_ps, gelw=gelw):
                            for j in range(2):
                                nc.tensor.matmul(
                                    bias_ps[:, j * L:(j + 1) * L],
                                    W2P_sb[:, dc, :],
                                    gelw[:, j * L:(j + 1) * L],
                                    start=(dc == 0),
                                    stop=(dc == NCH - 1),
                                )
                            if dc == NCH - 1:
                                finish_step(s)
                        pend.append(mm2)
                        if len(pend) > 2:
                            pend.popleft()()
                while pend:
                    pend.popleft()()
                # projections + padded-layout builds run in the gather tail
                for op in micro:
                    op()
                for h in range(H):
                    po = (h % 2) * DH
                    ch = h // 2
                    nc.sync.dma_start(qTp[0:DH, h, :], qT_sb[po:po + DH, ch, :])
                    nc.sync.dma_start(kTp[0:DH, h, :], kT_sb[po:po + DH, ch, :])

            # ---- phase 3: transposed attention, 2-stage head pipeline ----
            with (
                tc.tile_pool(name="p3b", bufs=40) as p3b,
                tc.tile_pool(name="p3e", bufs=38) as p3e,
                tc.tile_pool(name="p3r", bufs=2) as p3r,
                tc.tile_pool(name="p3o", bufs=2) as p3o,
                tc.tile_pool(name="lps", bufs=5, space="PSUM") as lps,
                tc.tile_pool(name="avps", bufs=2, space="PSUM") as avps,
                tc.tile_pool(name="rbps", bufs=1, space="PSUM") as rbps,
            ):
                bts = {}

                def load_bias(h, kcs):
                    for kc in kcs:
                        bt = p3b.tile([128, L], FP16, tag="bt",
                                      name=f"bt_{h}_{kc}")
                        eng = nc.sync if (h + kc) % 2 == 0 else nc.scalar
                        eng.dma_start(
                            bt[:],
                            fulls[kc][:, :, h, :].rearrange(
                                "c j q -> (c j) q"
                            ),
                        )
                        bts[(h, kc)] = bt

                def qkb(h, kc):
                    ps_l = lps.tile([128, L], F32, tag="lg",
                                    name=f"pl_{h}_{kc}")
                    nc.tensor.matmul(
                        ps_l[:],
                        kTp[:, h, kc * 128:(kc + 1) * 128],
                        qTp[:, h, :],
                        start=True,
                        stop=False,
                    )
                    nc.tensor.matmul(
                        ps_l[:],
                        ident[:],
                        bts.pop((h, kc))[:],
                        start=False,
                        stop=True,
                    )
                    exp_t = p3e.tile([128, L], BF16, tag="exp",
                                     name=f"ex_{h}_{kc}")
                    nc.scalar.activation(exp_t[:], ps_l[:], AF.Exp)
                    if dbg and h == 0 and kc == 0:
                        nc.sync.dma_start(dbg_exp[:, :], exp_t[:])
                    return exp_t

                exps = {}

                def stage1(h):
                    exps[h] = [qkb(h, kc) for kc in range(3)]

                def stage2(h):
                    av = avps.tile([DH + 1, L], F32, tag="av", name=f"av_{h}")
                    e = exps.pop(h)
                    for kc in range(3):
                        nc.tensor.matmul(
                            av[:], v_sb[:, kc, h, :], e[kc][:],
                            start=(kc == 0), stop=False,
                        )
                    e3 = qkb(h, 3)
                    nc.tensor.matmul(
                        av[:], v_sb[:, 3, h, :], e3[:],
                        start=False, stop=True,
                    )
                    rc = p3r.tile([1, L], BF16, tag="rc", name=f"rc_{h}")
                    with nc.allow_low_precision("bf16 softmax scale bcast"):
                        nc.vector.reciprocal(rc[:], av[DH:DH + 1, :])
                    rb = rbps.tile([DH, L], F32, tag="rb", name=f"rb_{h}")
                    nc.tensor.matmul(
                        rb[:], ones1[:], rc[:], start=True, stop=True,
                    )
                    rb_sb = p3r.tile([DH, L], F32, tag="rbs", name=f"rbs_{h}")
                    nc.scalar.activation(rb_sb[:], rb[:], AF.Copy)
                    nc.vector.tensor_tensor(
                        attnT[0:DH, h, :], av[0:DH, :], rb_sb[:], op=MULT
                    )

                for h in range(H):
                    load_bias(h, range(3))
                load_bias(0, [3])
                load_bias(1, [3])
                for h in range(H):
                    stage1(h)
                for h in range(H):
                    if h + 2 < H:
                        load_bias(h + 2, [3])
                    stage2(h)

                if dbg:
                    nc.sync.dma_start(dbg_full[:], fulls[0][:])
                    nc.sync.dma_start(dbg_qt[:], qT_sb[:])
                    nc.sync.dma_start(dbg_att[:], attnT[:])

                # ---- output projection ----
                for tc4 in range(4):
                    out_sb = p3o.tile([128, D], F32, tag="osb",
                                      name=f"osb_{tc4}")
                    for hf in range(2):
                        ps_o = lps.tile([128, L], F32, tag="lg",
                                        name=f"pso_{tc4}_{hf}")
                        sl = slice(hf * 384, (hf + 1) * 384)
                        for h2 in range(H):
                            nc.tensor.matmul(
                                ps_o[:, 0:384],
                                attnT[:, h2, tc4 * 128:(tc4 + 1) * 128],
                                Wo_sb[:, h2, sl],
                                start=(h2 == 0),
                                stop=(h2 == H - 1),
                            )
                        nc.vector.tensor_tensor(
                            out_sb[:, sl], ps_o[:, 0:384], bo_sb[:, sl], op=ADD
                        )
                    nc.sync.dma_start(
                        out_d[tc4 * 128:(tc4 + 1) * 128, :], out_sb[:]
                    )

    nc.compile()
    return nc


def _get_nc():
    if "nc" not in _CACHE:
        _CACHE["nc"] = _build()
    return _CACHE["nc"]


def _hi_lo(a, dt):
    hi = a.astype(dt)
    lo = (a - hi.astype(np.float32)).astype(dt)
    return hi, lo


def _owned_keys(c):
    # core c owns keys {g*128 + c*16 + j}, ordered by (g, j)
    ks = []
    for g in range(NCHUNK):
        for j in range(16):
            ks.append(g * 128 + c * 16 + j)
    return np.array(ks, dtype=np.int64)


def kernel(
    query,
    key_value,
    query_coords,
    key_coords,
    Wq,
    bq,
    Wk,
    bk,
    Wv,
    bv,
    Wo,
    bo,
    W1,
    b1,
    W2,
    b2,
):
    import ml_dtypes

    query = np.asarray(query, np.float32)
    key_value = np.asarray(key_value, np.float32)
    query_coords = np.asarray(query_coords, np.float32)
    key_coords = np.asarray(key_coords, np.float32)

    def chunked(w, dt=np.float16):  # [768, X] -> [128, 6, X]
        w = np.asarray(w, np.float32).astype(dt)
        return np.ascontiguousarray(w.reshape(NCH, 128, -1).transpose(1, 0, 2))

    def pchunk(b):  # [768] -> [128, 6]
        return np.ascontiguousarray(np.asarray(b, np.float32).reshape(NCH, 128).T)

    WqS = chunked(np.asarray(Wq, np.float32) * np.float32(SCALE))
    Wk_l = chunked(Wk)
    Wv_l = chunked(Wv, ml_dtypes.bfloat16)
    Wo_l = np.zeros((128, H, D), np.float16)
    Wo_l[0:DH] = (
        np.asarray(Wo, np.float32).astype(np.float16)
        .reshape(H, DH, D).transpose(1, 0, 2)
    )
    W2P_l = chunked(W2)  # [128, 6, 12] fp16
    W1f = np.asarray(W1, np.float32)
    W1hi, W1lo = _hi_lo(W1f, ml_dtypes.bfloat16)
    W1P = np.zeros((128, D), ml_dtypes.bfloat16)
    W1P[0:6] = W1hi
    W1P[6:12] = W1hi
    W1P[12:18] = W1lo
    W1P[18:24] = W1lo
    bqS = pchunk(np.asarray(bq, np.float32) * np.float32(SCALE))
    bk_l = pchunk(bk)
    b1_l = pchunk(b1)
    b2_l = np.ascontiguousarray(np.asarray(b2, np.float32).reshape(H, 1))
    bv_b = np.ascontiguousarray(np.broadcast_to(np.asarray(bv, np.float32), (128, D)))
    bo_b = np.ascontiguousarray(np.broadcast_to(np.asarray(bo, np.float32), (128, D)))

    in_maps = []
    for c in range(NCORES):
        ks = _owned_keys(c)
        # rel features for (owned k, all q): [64, 512, 6]
        delta = key_coords[ks][:, None, :] * -1.0 + query_coords[None, :, :]
        rel = np.concatenate([delta, np.abs(delta), np.square(delta)], axis=-1)
        relT = rel.reshape(KS * L, 6).T  # [6, 64*512] (k outer, q inner)
        rhi, rlo = _hi_lo(relT, ml_dtypes.bfloat16)
        relP = np.zeros((128, KS * L), ml_dtypes.bfloat16)
        relP[0:6] = rhi
        relP[6:12] = rlo
        relP[12:18] = rhi
        relP[18:24] = rlo
        in_maps.append(
            {
                "xqT": np.ascontiguousarray(query[c].T).astype(np.float16),
                "kvT": np.ascontiguousarray(key_value[c].T).astype(np.float16),
                "kvTB": np.ascontiguousarray(key_value[c].T).astype(
                    ml_dtypes.bfloat16
                ),
                "relP": relP,
                "WqS": WqS,
                "Wk": Wk_l,
                "Wv": Wv_l,
                "Wo": Wo_l,
                "W1P": W1P,
                "W2P": W2P_l,
                "bqS": bqS,
                "bk": bk_l,
                "b1": b1_l,
                "b2": b2_l,
                "bvb": bv_b,
                "bob": bo_b,
                "idn": np.eye(128, dtype=np.float16),
                "zer": np.zeros((64, H * L), dtype=np.float16),
                "one": np.ones((1, 64), dtype=ml_dtypes.bfloat16),
                "oneh": np.ones((128, 4 * H), dtype=ml_dtypes.bfloat16),
            }
        )

    nc = _get_nc()
    res = bass_utils.run_bass_kernel_spmd(nc, in_maps, core_ids=list(range(NCORES)))
    out = np.stack([res.results[c]["out"] for c in range(NCORES)], axis=0)
    return out.astype(np.float32)


# revision 54
# speedup vs baseline: 1.1603x; 1.1603x over previous
"""Cross-attention with relative-position-bias MLP on 8 Trainium2 NeuronCores.

Sharding: batch-parallel attention (core c owns batch element c) +
k-sharded bias MLP: core c computes bias rows for keys
{g*64 + c*8 + j : g<8, j<8} so that the AllGather of chunk g yields a
DRAM layout [c, j, h, q] whose flattened (c, j) order IS the global key
order of the 64-key block g.  Phase 3 runs attention fully transposed
(logits as [k, q]): bias tiles stream in with 2KB descriptors (two
64-partition halves per 128-k tile) and are folded into the logits PSUM
via an fp16 identity-matmul accumulate; exp reads PSUM directly (bf16
output - range safe without max subtraction); AV consumes exp with V in
natural [k, dh] layout plus a ones column that yields the softmax sums
for free; per-head normalization is a bf16 reciprocal + K=1 bf16
broadcast matmul + one [64,512] multiply.

Precision: 16-bit everywhere on the PE (1 cyc/row; f32r lowers to
2-pass fp32 HIGH mode on this stack), fp32 PSUM accumulation. The bias
MLP mm1 uses bf16 hi/lo packed into K=24 (exact to ~2^-17).

Scheduling: phase-1 mm2 runs two dc slots late (software pipeline) and
phase-3 projections interleave one matmul per dc slot so the PE never
idles on the gelu chain (frequent short stalls halve the PE clock);
phase-3 heads run in a 2-stage pipeline so QK work of head h+1 covers
the exp latency of head h. Big weight loads ride the ACT/Pool DMA
queues so the Sync queue serves the latency-critical rel/shard/bias
tiles.

Self-contained: hardcodes all shapes; builds/compiles the Bass kernel on
first call and runs it via bass_utils.run_bass_kernel_spmd on cores 0-7.
"""

import numpy as np

import concourse.bass as bass
import concourse.mybir as mybir
import concourse.tile as tile
from concourse import bacc, bass_utils

F32 = mybir.dt.float32
F32R = mybir.dt.float32r
BF16 = mybir.dt.bfloat16
FP16 = mybir.dt.float16
AF = mybir.ActivationFunctionType
ADD = mybir.AluOpType.add
MULT = mybir.AluOpType.mult

NCORES = 8
B = 8
L = 512
D = 768
H = 12
DH = 64
NCH = D // 128
KS = L // NCORES          # 64 owned keys per core
NSTEP = KS // 2           # 32 phase-1 steps (2 owned keys per step)
NCHUNK = 4                # all-gather chunks (16 owned keys each)
SPC = NSTEP // NCHUNK     # steps per chunk = 4
SCALE = DH ** -0.5

_CACHE = {}


def _build(dbg=False):
    nc = bacc.Bacc("TRN2", target_bir_lowering=False, debug=False, num_devices=NCORES)

    xqT_d = nc.dram_tensor("xqT", [D, L], FP16, kind="ExternalInput")
    kvT_d = nc.dram_tensor("kvT", [D, L], FP16, kind="ExternalInput")
    kvTB_d = nc.dram_tensor("kvTB", [D, L], BF16, kind="ExternalInput")
    relP_d = nc.dram_tensor("relP", [128, NSTEP * 2 * L], BF16, kind="ExternalInput")
    WqS_d = nc.dram_tensor("WqS", [128, NCH, D], FP16, kind="ExternalInput")
    Wk_d = nc.dram_tensor("Wk", [128, NCH, D], FP16, kind="ExternalInput")
    Wv_d = nc.dram_tensor("Wv", [128, NCH, D], BF16, kind="ExternalInput")
    Wo_d = nc.dram_tensor("Wo", [128, H, D], FP16, kind="ExternalInput")
    W1P_d = nc.dram_tensor("W1P", [128, D], BF16, kind="ExternalInput")
    W2P_d = nc.dram_tensor("W2P", [128, NCH, H], FP16, kind="ExternalInput")
    bqS_d = nc.dram_tensor("bqS", [128, NCH], F32, kind="ExternalInput")
    bk_d = nc.dram_tensor("bk", [128, NCH], F32, kind="ExternalInput")
    b1_d = nc.dram_tensor("b1", [128, NCH], F32, kind="ExternalInput")
    b2_d = nc.dram_tensor("b2", [H, 1], F32, kind="ExternalInput")
    bv_d = nc.dram_tensor("bvb", [128, D], F32, kind="ExternalInput")
    bo_d = nc.dram_tensor("bob", [128, D], F32, kind="ExternalInput")
    idn_d = nc.dram_tensor("idn", [128, 128], FP16, kind="ExternalInput")
    one_d = nc.dram_tensor("one", [1, 64], BF16, kind="ExternalInput")
    zer_d = nc.dram_tensor("zer", [64, H * L], FP16, kind="ExternalInput")
    oneh_d = nc.dram_tensor("oneh", [128, 4 * H], BF16, kind="ExternalInput")
    out_d = nc.dram_tensor("out", [L, D], F32, kind="ExternalOutput")
    if dbg:
        dbg_full = nc.dram_tensor(
            "dbg_full", [NCORES, 16, H, L], FP16, kind="ExternalOutput"
        )
        dbg_qt = nc.dram_tensor("dbg_qt", [128, NCH, L], FP16, kind="ExternalOutput")
        dbg_exp = nc.dram_tensor("dbg_exp", [128, L], BF16, kind="ExternalOutput")
        dbg_att = nc.dram_tensor("dbg_att", [DH, H, L], FP16, kind="ExternalOutput")

    with tile.TileContext(nc) as tc:
        with (
            tc.tile_pool(name="dram", bufs=1, space="DRAM") as dpool,
            tc.tile_pool(name="persist", bufs=1) as pp,
            tc.tile_pool(name="p3b", bufs=48) as p3b,
        ):
            shards = [
                dpool.tile([16, H, L], FP16, name=f"shard{g}") for g in range(NCHUNK)
            ]
            fulls = [
                dpool.tile([NCORES, 16, H, L], FP16, name=f"full{g}",
                           addr_space="Shared")
                for g in range(NCHUNK)
            ]

            # ---- persistent SBUF (phase-1-critical small tiles on sync) ----
            W1p_sb = pp.tile([128, D], BF16, name="W1p_sb")
            nc.sync.dma_start(W1p_sb[:], W1P_d[:, :])
            W2P_sb = pp.tile([128, NCH, H], FP16, name="W2P_sb")
            nc.sync.dma_start(W2P_sb[:], W2P_d[:, :, :])
            b1_sb = pp.tile([128, NCH], F32, name="b1_sb")
            nc.sync.dma_start(b1_sb[:], b1_d[:, :])
            b2_sb = pp.tile([H, 1], F32, name="b2_sb")
            nc.sync.dma_start(b2_sb[:], b2_d[:, :])
            # rest of the constants ride the ACT-engine DMA queue
            bq_sb = pp.tile([128, NCH], F32, name="bq_sb")
            nc.scalar.dma_start(bq_sb[:], bqS_d[:, :])
            bk_sb = pp.tile([128, NCH], F32, name="bk_sb")
            nc.scalar.dma_start(bk_sb[:], bk_d[:, :])
            bv_sb = pp.tile([128, D], F32, name="bv_sb")
            nc.scalar.dma_start(bv_sb[:], bv_d[:, :])
            bo_sb = pp.tile([128, D], F32, name="bo_sb")
            nc.scalar.dma_start(bo_sb[:], bo_d[:, :])
            ident = pp.tile([128, 128], FP16, name="ident")
            nc.scalar.dma_start(ident[:], idn_d[:, :])
            ones1 = pp.tile([1, DH], BF16, name="ones1")
            nc.scalar.dma_start(ones1[:], one_d[0:1, :])

            qT_sb = pp.tile([128, NCH, L], FP16, name="qT_sb")
            kT_sb = pp.tile([128, NCH, L], FP16, name="kT_sb")
            # V in natural [k, dh] layout, 65 slots per head (slot 64 = ones)
            v_sb = pp.tile([128, 4, H, DH + 1], BF16, name="v_sb")
            oneh_sb = pp.tile([128, 4 * H], BF16, name="oneh_sb")
            nc.scalar.dma_start(oneh_sb[:], oneh_d[:, :])
            nc.scalar.activation(
                v_sb[:, :, :, DH:DH + 1].rearrange("p g h o -> p g (h o)"),
                oneh_sb[:].rearrange("p (g h) -> p g h", g=4),
                AF.Copy,
            )
            attnT = pp.tile([128, H, L], FP16, name="attnT")
            nc.scalar.dma_start(attnT[DH:128, :, :], zer_d.ap().rearrange(
                "p (h q) -> p h q", h=H))
            qTp = pp.tile([128, H, L], FP16, name="qTp")
            nc.scalar.dma_start(qTp[DH:128, :, :], zer_d.ap().rearrange(
                "p (h q) -> p h q", h=H))
            kTp = pp.tile([128, H, L], FP16, name="kTp")
            nc.scalar.dma_start(kTp[DH:128, :, :], zer_d.ap().rearrange(
                "p (h q) -> p h q", h=H))
            Wo_sb = pp.tile([128, H, D], FP16, name="Wo_sb")
            nc.scalar.dma_start(Wo_sb[:], Wo_d[:, :, :])

            # bias tiles for phase 3: loaded on the gpsimd queue right
            # after each chunk's all-gather, overlapped with phase 1
            bts = {}

            def load_bias_chunk(g):
                for h in range(H):
                    bt = p3b.tile([128, L], FP16, tag="bt", name=f"bt_{h}_{g}")
                    nc.gpsimd.dma_start(
                        bt[:],
                        fulls[g][:, :, h, :].rearrange("c j q -> (c j) q"),
                    )
                    bts[(h, g)] = bt

            # ---- phase 1 + interleaved projections ----
            with (
                tc.tile_pool(name="p1w", bufs=1) as p1w,
                tc.tile_pool(name="p1rel", bufs=5) as p1rel,
                tc.tile_pool(name="p1gel", bufs=4) as p1gel,
                tc.tile_pool(name="p1out", bufs=3) as p1out,
                tc.tile_pool(name="p1ps", bufs=2, space="PSUM") as p1ps,
                tc.tile_pool(name="p1psb", bufs=1, space="PSUM") as p1psb,
                tc.tile_pool(name="p1pj", bufs=2, space="PSUM") as p1pj,
            ):
                # activation-engine DMA queue: big projection operands
                WqS_sb = p1w.tile([128, NCH, D], FP16, name="WqS_sb")
                nc.scalar.dma_start(WqS_sb[:], WqS_d[:, :, :])
                xqT_sb = p1w.tile([128, NCH, L], FP16, name="xqT_sb")
                nc.scalar.dma_start(
                    xqT_sb[:], xqT_d.ap().rearrange("(c p) t -> p c t", p=128)
                )
                Wk_sb = p1w.tile([128, NCH, D], FP16, name="Wk_sb")
                nc.scalar.dma_start(Wk_sb[:], Wk_d[:, :, :])
                kvT_sb = p1w.tile([128, NCH, L], FP16, name="kvT_sb")
                nc.scalar.dma_start(
                    kvT_sb[:], kvT_d.ap().rearrange("(c p) t -> p c t", p=128)
                )
                Wv_sb = p1w.tile([128, NCH, D], BF16, name="Wv_sb")
                nc.scalar.dma_start(Wv_sb[:], Wv_d[:, :, :])
                kvTb_sb = p1w.tile([128, NCH, L], BF16, name="kvTb_sb")
                nc.scalar.dma_start(
                    kvTb_sb[:], kvTB_d.ap().rearrange("(c p) t -> p c t", p=128)
                )

                def proj_microops():
                    # q / k projections: out [128 (oc dims), 512]
                    for W_sb, x_sb, b_sb, out_t, pfx in (
                        (WqS_sb, xqT_sb, bq_sb, qT_sb, "q"),
                        (Wk_sb, kvT_sb, bk_sb, kT_sb, "k"),
                    ):
                        for oc in range(NCH):
                            cell = {}

                            def mk_mm(di, cell=cell, W_sb=W_sb, x_sb=x_sb,
                                      oc=oc, pfx=pfx):
                                def op():
                                    if di == 0:
                                        cell["ps"] = p1pj.tile(
                                            [128, L], F32, tag="pj",
                                            name=f"pp{pfx}_{oc}",
                                        )
                                    nc.tensor.matmul(
                                        cell["ps"][:],
                                        W_sb[:, di, oc * 128:(oc + 1) * 128],
                                        x_sb[:, di, :],
                                        start=(di == 0),
                                        stop=(di == NCH - 1),
                                    )
                                return op
                            for di in range(NCH):
                                yield mk_mm(di)

                            def fin(cell=cell, b_sb=b_sb, out_t=out_t, oc=oc):
                                nc.vector.tensor_scalar_add(
                                    out_t[:, oc, :], cell["ps"][:],
                                    b_sb[:, oc:oc + 1],
                                )
                            yield fin
                    # v projection: out [128 k-chunk, 6 heads x 64]
                    for tc4 in range(4):
                        for hf in range(2):
                            cell = {}

                            def mk_mm(di, cell=cell, tc4=tc4, hf=hf):
                                def op():
                                    if di == 0:
                                        cell["ps"] = p1pj.tile(
                                            [128, L], F32, tag="pj",
                                            name=f"ppv_{tc4}_{hf}",
                                        )
                                    nc.tensor.matmul(
                                        cell["ps"][:, 0:384],
                                        kvTb_sb[:, di,
                                                tc4 * 128:(tc4 + 1) * 128],
                                        Wv_sb[:, di, hf * 384:(hf + 1) * 384],
                                        start=(di == 0),
                                        stop=(di == NCH - 1),
                                    )
                                return op
                            for di in range(NCH):
                                yield mk_mm(di)

                            def fin(cell=cell, tc4=tc4, hf=hf):
                                nc.vector.tensor_tensor(
                                    v_sb[:, tc4, 6 * hf:6 * (hf + 1), 0:DH],
                                    cell["ps"][:, 0:384].rearrange(
                                        "p (h d) -> p h d", h=6
                                    ),
                                    bv_sb[
                                        :, hf * 384:(hf + 1) * 384
                                    ].rearrange("p (h d) -> p h d", h=6),
                                    op=ADD,
                                )
                            yield fin
                micro = proj_microops()
                micro_done = False

                from collections import deque
                pend = deque()
                bias_tiles = {}

                def finish_step(s):
                    g = s // SPC
                    bias_ps = bias_tiles.pop(s)
                    bsb = p1out.tile([H, 2 * L], FP16, tag="bsb",
                                     name=f"bsb_{s}")
                    nc.vector.tensor_scalar_add(bsb[:], bias_ps[:], b2_sb[:, 0:1])
                    j0 = (2 * s) % 16
                    nc.sync.dma_start(
                        shards[g][j0:j0 + 2, :, :].rearrange("k h q -> h k q"),
                        bsb[:].rearrange("h (k q) -> h k q", k=2),
                    )
                    if s % SPC == SPC - 1:
                        nc.gpsimd.collective_compute(
                            "AllGather",
                            mybir.AluOpType.bypass,
                            replica_groups=[list(range(NCORES))],
                            ins=[shards[g][:].opt()],
                            outs=[fulls[g][:].opt()],
                        )
                        load_bias_chunk(g)

                rels = {}

                def load_rel(s):
                    if s >= NSTEP:
                        return
                    rel2 = p1rel.tile([128, 2 * L], BF16, tag="rel",
                                      name=f"rel_{s}")
                    nc.sync.dma_start(
                        rel2[:], relP_d[:, s * 2 * L:(s + 1) * 2 * L]
                    )
                    rels[s] = rel2

                for s in range(4):
                    load_rel(s)
                for s in range(NSTEP):
                    load_rel(s + 4)
                    rel2 = rels.pop(s)
                    bias_ps = p1psb.tile([H, 2 * L], F32, tag="bps",
                                         name=f"bps_{s}")
                    bias_tiles[s] = bias_ps
                    for dc in range(NCH):
                        hid = p1ps.tile([128, 2 * L], F32, tag="hid",
                                        name=f"hid_{s}_{dc}")
                        for j in range(2):
                            nc.tensor.matmul(
                                hid[:, j * L:(j + 1) * L],
                                W1p_sb[:, dc * 128:(dc + 1) * 128],
                                rel2[:, j * L:(j + 1) * L],
                                start=True,
                                stop=True,
                            )
                        gelw = p1gel.tile([128, 2 * L], FP16, tag="gel",
                                          name=f"gel_{s}_{dc}")
                        nc.scalar.activation(
                            gelw[:], hid[:], AF.Gelu, bias=b1_sb[:, dc:dc + 1]
                        )

                        def mm2(s=s, dc=dc, bias_ps=bias_ps, gelw=gelw):
                            for j in range(2):
                                nc.tensor.matmul(
                                    bias_ps[:, j * L:(j + 1) * L],
                                    W2P_sb[:, dc, :],
                                    gelw[:, j * L:(j + 1) * L],
                                    start=(dc == 0),
                                    stop=(dc == NCH - 1),
                                )
                            if dc == NCH - 1:
                                finish_step(s)
                        pend.append(mm2)
                        if len(pend) > 2:
                            pend.popleft()()
                while pend:
                    pend.popleft()()
                # projections + padded-layout builds run in the gather tail
                for op in micro:
                    op()
                for h in range(H):
                    po = (h % 2) * DH
                    ch = h // 2
                    nc.sync.dma_start(qTp[0:DH, h, :], qT_sb[po:po + DH, ch, :])
                    nc.sync.dma_start(kTp[0:DH, h, :], kT_sb[po:po + DH, ch, :])

            # ---- phase 3: transposed attention, 2-stage head pipeline ----
            with (
                tc.tile_pool(name="p3e", bufs=38) as p3e,
                tc.tile_pool(name="p3r", bufs=2) as p3r,
                tc.tile_pool(name="p3o", bufs=2) as p3o,
                tc.tile_pool(name="lps", bufs=5, space="PSUM") as lps,
                tc.tile_pool(name="avps", bufs=2, space="PSUM") as avps,
                tc.tile_pool(name="rbps", bufs=1, space="PSUM") as rbps,
            ):
                def qkb(h, kc):
                    ps_l = lps.tile([128, L], F32, tag="lg",
                                    name=f"pl_{h}_{kc}")
                    nc.tensor.matmul(
                        ps_l[:],
                        kTp[:, h, kc * 128:(kc + 1) * 128],
                        qTp[:, h, :],
                        start=True,
                        stop=False,
                    )
                    nc.tensor.matmul(
                        ps_l[:],
                        ident[:],
                        bts.pop((h, kc))[:],
                        start=False,
                        stop=True,
                    )
                    exp_t = p3e.tile([128, L], BF16, tag="exp",
                                     name=f"ex_{h}_{kc}")
                    nc.scalar.activation(exp_t[:], ps_l[:], AF.Exp)
                    if dbg and h == 0 and kc == 0:
                        nc.sync.dma_start(dbg_exp[:, :], exp_t[:])
                    return exp_t

                exps = {}

                def stage1(h):
                    exps[h] = [qkb(h, kc) for kc in range(3)]

                def stage2(h):
                    av = avps.tile([DH + 1, L], F32, tag="av", name=f"av_{h}")
                    e = exps.pop(h)
                    for kc in range(3):
                        nc.tensor.matmul(
                            av[:], v_sb[:, kc, h, :], e[kc][:],
                            start=(kc == 0), stop=False,
                        )
                    e3 = qkb(h, 3)
                    nc.tensor.matmul(
                        av[:], v_sb[:, 3, h, :], e3[:],
                        start=False, stop=True,
                    )
                    rc = p3r.tile([1, L], BF16, tag="rc", name=f"rc_{h}")
                    with nc.allow_low_precision("bf16 softmax scale bcast"):
                        nc.vector.reciprocal(rc[:], av[DH:DH + 1, :])
                    rb = rbps.tile([DH, L], F32, tag="rb", name=f"rb_{h}")
                    nc.tensor.matmul(
                        rb[:], ones1[:], rc[:], start=True, stop=True,
                    )
                    rb_sb = p3r.tile([DH, L], F32, tag="rbs", name=f"rbs_{h}")
                    nc.scalar.activation(rb_sb[:], rb[:], AF.Copy)
                    nc.vector.tensor_tensor(
                        attnT[0:DH, h, :], av[0:DH, :], rb_sb[:], op=MULT
                    )

                for h in range(H):
                    stage1(h)
                for h in range(H):
                    stage2(h)

                if dbg:
                    nc.sync.dma_start(dbg_full[:], fulls[0][:])
                    nc.sync.dma_start(dbg_qt[:], qT_sb[:])
                    nc.sync.dma_start(dbg_att[:], attnT[:])

                # ---- output projection ----
                for tc4 in range(4):
                    out_sb = p3o.tile([128, D], F32, tag="osb",
                                      name=f"osb_{tc4}")
                    for hf in range(2):
                        ps_o = lps.tile([128, L], F32, tag="lg",
                                        name=f"pso_{tc4}_{hf}")
                        sl = slice(hf * 384, (hf + 1) * 384)
                        for h2 in range(H):
                            nc.tensor.matmul(
                                ps_o[:, 0:384],
                                attnT[:, h2, tc4 * 128:(tc4 + 1) * 128],
                                Wo_sb[:, h2, sl],
                                start=(h2 == 0),
                                stop=(h2 == H - 1),
                            )
                        nc.vector.tensor_tensor(
                            out_sb[:, sl], ps_o[:, 0:384], bo_sb[:, sl], op=ADD
                        )
                    nc.sync.dma_start(
                        out_d[tc4 * 128:(tc4 + 1) * 128, :], out_sb[:]
                    )

    nc.compile()
    return nc


def _get_nc():
    if "nc" not in _CACHE:
        _CACHE["nc"] = _build()
    return _CACHE["nc"]


def _hi_lo(a, dt):
    hi = a.astype(dt)
    lo = (a - hi.astype(np.float32)).astype(dt)
    return hi, lo


def _owned_keys(c):
    # core c owns keys {g*128 + c*16 + j}, ordered by (g, j)
    ks = []
    for g in range(NCHUNK):
        for j in range(16):
            ks.append(g * 128 + c * 16 + j)
    return np.array(ks, dtype=np.int64)


def kernel(
    query,
    key_value,
    query_coords,
    key_coords,
    Wq,
    bq,
    Wk,
    bk,
    Wv,
    bv,
    Wo,
    bo,
    W1,
    b1,
    W2,
    b2,
):
    import ml_dtypes

    query = np.asarray(query, np.float32)
    key_value = np.asarray(key_value, np.float32)
    query_coords = np.asarray(query_coords, np.float32)
    key_coords = np.asarray(key_coords, np.float32)

    def chunked(w, dt=np.float16):  # [768, X] -> [128, 6, X]
        w = np.asarray(w, np.float32).astype(dt)
        return np.ascontiguousarray(w.reshape(NCH, 128, -1).transpose(1, 0, 2))

    def pchunk(b):  # [768] -> [128, 6]
        return np.ascontiguousarray(np.asarray(b, np.float32).reshape(NCH, 128).T)

    WqS = chunked(np.asarray(Wq, np.float32) * np.float32(SCALE))
    Wk_l = chunked(Wk)
    Wv_l = chunked(Wv, ml_dtypes.bfloat16)
    Wo_l = np.zeros((128, H, D), np.float16)
    Wo_l[0:DH] = (
        np.asarray(Wo, np.float32).astype(np.float16)
        .reshape(H, DH, D).transpose(1, 0, 2)
    )
    W2P_l = chunked(W2)  # [128, 6, 12] fp16
    W1f = np.asarray(W1, np.float32)
    W1hi, W1lo = _hi_lo(W1f, ml_dtypes.bfloat16)
    W1P = np.zeros((128, D), ml_dtypes.bfloat16)
    W1P[0:6] = W1hi
    W1P[6:12] = W1hi
    W1P[12:18] = W1lo
    W1P[18:24] = W1lo
    bqS = pchunk(np.asarray(bq, np.float32) * np.float32(SCALE))
    bk_l = pchunk(bk)
    b1_l = pchunk(b1)
    b2_l = np.ascontiguousarray(np.asarray(b2, np.float32).reshape(H, 1))
    bv_b = np.ascontiguousarray(np.broadcast_to(np.asarray(bv, np.float32), (128, D)))
    bo_b = np.ascontiguousarray(np.broadcast_to(np.asarray(bo, np.float32), (128, D)))

    in_maps = []
    for c in range(NCORES):
        ks = _owned_keys(c)
        # rel features for (owned k, all q): [64, 512, 6]
        delta = key_coords[ks][:, None, :] * -1.0 + query_coords[None, :, :]
        rel = np.concatenate([delta, np.abs(delta), np.square(delta)], axis=-1)
        relT = rel.reshape(KS * L, 6).T  # [6, 64*512] (k outer, q inner)
        rhi, rlo = _hi_lo(relT, ml_dtypes.bfloat16)
        relP = np.zeros((128, KS * L), ml_dtypes.bfloat16)
        relP[0:6] = rhi
        relP[6:12] = rlo
        relP[12:18] = rhi
        relP[18:24] = rlo
        in_maps.append(
            {
                "xqT": np.ascontiguousarray(query[c].T).astype(np.float16),
                "kvT": np.ascontiguousarray(key_value[c].T).astype(np.float16),
                "kvTB": np.ascontiguousarray(key_value[c].T).astype(
                    ml_dtypes.bfloat16
                ),
                "relP": relP,
                "WqS": WqS,
                "Wk": Wk_l,
                "Wv": Wv_l,
                "Wo": Wo_l,
                "W1P": W1P,
                "W2P": W2P_l,
                "bqS": bqS,
                "bk": bk_l,
                "b1": b1_l,
                "b2": b2_l,
                "bvb": bv_b,
                "bob": bo_b,
                "idn": np.eye(128, dtype=np.float16),
                "zer": np.zeros((64, H * L), dtype=np.float16),
                "one": np.ones((1, 64), dtype=ml_dtypes.bfloat16),
                "oneh": np.ones((128, 4 * H), dtype=ml_dtypes.bfloat16),
            }
        )

    nc = _get_nc()
    res = bass_utils.run_bass_kernel_spmd(nc, in_maps, core_ids=list(range(NCORES)))
    out = np.stack([res.results[c]["out"] for c in range(NCORES)], axis=0)
    return out.astype(np.float32)
